# revision 1
# baseline (speedup 1.0000x reference)
"""ColorAttention Trainium2 kernel.

Data-parallel over batch: core b handles batch element b.
Per core:
  - mask [256,256,313] is cast to fp8 (0/1 values, lossless) on the host and
    streamed from HBM (20.9MB with c padded to 320), then patch-reduced via
    block-diagonal ones matmuls on the PE in fp8 DoubleRow mode (2 image
    columns per PE cycle, PSUM accumulation), giving m[s,c] = sum over 16x16
    patch. Multiplicative attention mask is_one(m) = relu(1-(m-1)^2)
    (exact for integer m: 1 iff m==1).
  - attention computed in transposed layout throughout:
      qkvT[f,n] = sum_e qkv_wT[e,f] * inputsT[e,n]
      scoresT[m,n] = sum_d kT[d,m] qT[d,n];  expT = exp(scoresT/tau) * mask
      outT_aug[d|1,n] = sum_m v_aug[m,d|1] expT[m,n]   (row 64 = denom)
      out[n,g] = (sum_{h,d} (outT_h/denom_h)[d,n] o_wT[h*64+d,g]) + o_b
  - all f32 matmuls use float32r moving operands (1 cyc/col at N>=256)
  - heads are packed in pairs on the 128 partitions for the normalize /
    o_proj stages so DVE works full-width and the PE overlaps the two
    64-contraction row groups.
"""

import os
import numpy as np
import ml_dtypes

# tolerate environments without the optional NTFF profile hook module when
# BASS_TRACE is set externally
try:
    import antenv.axon_hooks  # noqa: F401
except Exception:
    import sys as _sys
    import types as _types
    _m = _types.ModuleType("antenv.axon_hooks")
    _m.set_axon_ntff_profile_hook = lambda h: None
    _m.get_axon_ntff_profile_hook = lambda: None
    try:
        import antenv
        antenv.axon_hooks = _m
        _sys.modules["antenv.axon_hooks"] = _m
    except Exception:
        pass

import concourse.bass as bass
import concourse.mybir as mybir
import concourse.tile as tile
from concourse import bacc
from concourse.bass_utils import run_bass_kernel_spmd

F32 = mybir.dt.float32
F32R = mybir.dt.float32r
BF16 = mybir.dt.bfloat16
FP8 = mybir.dt.float8e4
AFT = mybir.ActivationFunctionType
DR = mybir.MatmulPerfMode.DoubleRow
USE_DR = os.environ.get("KERNEL_NO_DR", "") != "1"  # bisect switch

B = 8
SEQ = 256
NCLS = 313
NCP = 320  # c dim padded to a 16B multiple so fp8 DoubleRow strides are legal
E = 512
NH = 8
HD = 64
N1 = SEQ + NCLS  # 569
P = 16
IMG = 256

# n/m chunking of the 569 token dim.
# fp32r matmuls require even free counts, so padded widths (..P) are used for
# fp32r operands/psum; real widths for bf16 ops and final stores.
N1P = 570
CHUNKS = [(0, 128), (128, 128), (256, 128), (384, 128), (512, 57)]
CWP = [128, 128, 128, 128, 58]
SPANS = [(0, 512), (512, 58)]

LAST_RESULT = None
_CACHED = {}


def r32(ap):
    if ap.dtype == F32R:
        return ap
    return ap.bitcast(F32R)


def _build_program():
    nc = bacc.Bacc("TRN2", target_bir_lowering=False, debug=False, num_devices=B)

    # ---- DRAM I/O ----
    d_xT = nc.dram_tensor("xT", [128, 4 * N1P], BF16, kind="ExternalInput").ap()
    d_mask = nc.dram_tensor("mask", [IMG, IMG * NCP], FP8, kind="ExternalInput").ap()
    d_qkvwT = nc.dram_tensor("qkv_wT", [128, 4 * 3 * E], BF16, kind="ExternalInput").ap()
    d_owT = nc.dram_tensor("o_wT", [64, 8 * E], BF16, kind="ExternalInput").ap()
    d_ob = nc.dram_tensor("o_b", [1, E], F32, kind="ExternalInput").ap()
    d_tau = nc.dram_tensor("tau", [1, 1], F32, kind="ExternalInput").ap()
    d_bd = nc.dram_tensor("bd", [16, 128, 2 * 128], FP8, kind="ExternalInput").ap()
    d_ident = nc.dram_tensor("ident", [128, 128], BF16, kind="ExternalInput").ap()
    d_unitv = nc.dram_tensor("unitv", [128, 512], F32R, kind="ExternalInput").ap()
    d_out = nc.dram_tensor("out", [N1, E], BF16, kind="ExternalOutput").ap()

    with tile.TileContext(nc) as tc:
        _emit(nc, tc, d_xT, d_mask, d_qkvwT, d_owT, d_ob, d_tau, d_bd, d_ident, d_unitv, d_out)

    nc.compile()
    return nc


def _emit(nc, tc, d_xT, d_mask, d_qkvwT, d_owT, d_ob, d_tau, d_bd, d_ident, d_unitv, d_out):
    from contextlib import ExitStack

    ctx = ExitStack()
    singles = ctx.enter_context(tc.tile_pool(name="singles", bufs=1))
    expool = ctx.enter_context(tc.tile_pool(name="expT", bufs=40))
    opool = ctx.enter_context(tc.tile_pool(name="outTsb", bufs=8))
    spool = ctx.enter_context(tc.tile_pool(name="smalls", bufs=2))
    # psum pools: ps_a = 1-bank tiles (qkv/v/warmup/o_proj), ps_big = 2-bank
    # [*, 570] tiles (scores exp, attn@v, denom broadcast). ps_mask scoped to
    # the stream phase; ps_out (denominator gather) opens after it closes.
    ps_a = ctx.enter_context(tc.tile_pool(name="ps_a", bufs=2, space="PSUM"))
    ps_big = ctx.enter_context(tc.tile_pool(name="ps_big", bufs=2, space="PSUM"))
    mctx = ExitStack()
    mpool = mctx.enter_context(tc.tile_pool(name="mask_stream", bufs=4))
    ps_mask = mctx.enter_context(tc.tile_pool(name="ps_mask", bufs=2, space="PSUM"))

    # ---- persistent SBUF ----
    inputsT4 = singles.tile([128, 4, N1P], BF16, tag="inT", name="inputsT4")
    inputsT = [inputsT4[:, i, :] for i in range(4)]
    qkvwT4 = singles.tile([128, 4, 3 * E], BF16, tag="qkvwT", name="qkvwT4")
    qkvwT = [qkvwT4[:, i, :] for i in range(4)]
    owT8 = singles.tile([64, 8, E], BF16, tag="owT", name="owT8")
    owT = [owT8[:, h, :] for h in range(8)]
    bd_sb = singles.tile([128, 16, 2, 128], FP8, tag="bd", name="bd_sb")
    ident_sb = singles.tile([128, 128], BF16, tag="ident", name="ident_sb")
    ones_sb = singles.tile([128, 64], F32R, tag="ones", name="ones_sb")
    unitv_sb = singles.tile([128, 512], F32R, tag="unitv", name="unitv_sb")
    rtau = singles.tile([128, 1], F32, tag="rtau", name="rtau")
    ob_bc = singles.tile([128, E], F32, tag="ob", name="ob_bc")
    qkT = [singles.tile([128, N1P], F32R, tag=f"qkT{i}", name=f"qkT{i}") for i in range(8)]
    v_sb = [singles.tile([128, NH * (HD + 1)], BF16, tag=f"vsb{i}", name=f"v_sb{i}") for i in range(5)]
    isone = [singles.tile([128, NCLS], BF16, tag=f"iso{i}", name=f"isone{i}") for i in range(2)]
    isoT = [singles.tile([128, SEQ], BF16, tag=f"isoT{i}", name=f"isoT{i}") for i in range(3)]

    # ---- short HAM warmup: keep the PE busy while mask tile 0 lands ----
    scr = singles.tile([128, 640], BF16, tag="scr", name="scr")
    nc.vector.memset(scr, 1.0)
    ps_warm = ps_a.tile([128, 512], F32, tag="psa", name="ps_warm")
    for _ in range(8):
        nc.tensor.matmul(out=ps_warm, lhsT=scr[:, 0:128], rhs=scr[:, 128:640],
                         start=True, stop=True)

    # ---- mask stream tiles: pre-issue the first three DMAs on the SP HWDGE
    # ring so the PE reduce starts early; all other setup DMAs go on the ACT
    # HWDGE ring so they don't queue ahead of the mask stream ----
    n_tiles = 8  # per row-block; each tile spans 2 w groups (32 image cols)
    ROWS_PER_TILE = 128
    COLS_PER_TILE = 32

    def mask_tile_dma(rt, T):
        t = mpool.tile([128, 2, 8, 2, NCP], FP8, tag="mstream", name="mstream")
        src = bass.AP(
            tensor=d_mask.tensor,
            offset=d_mask.offset + rt * ROWS_PER_TILE * IMG * NCP
            + T * COLS_PER_TILE * NCP,
            ap=[[IMG * NCP, 128], [1, COLS_PER_TILE * NCP]],
        )
        nc.gpsimd.dma_start(out=t, in_=src)
        return t

    pre = {}
    for T in range(3):
        pre[(0, T)] = mask_tile_dma(0, T)

    # ---- setup DMAs: host pre-packs each tensor into its exact SBUF
    # layout so each is one transfer with long contiguous per-partition
    # lines (ACT HWDGE ring, separate from the mask stream) ----
    nc.scalar.dma_start(out=inputsT4, in_=d_xT)
    nc.scalar.dma_start(out=qkvwT4, in_=d_qkvwT)
    nc.scalar.dma_start(out=owT8, in_=d_owT)
    src_bd = bass.AP(tensor=d_bd.tensor, offset=d_bd.offset,
                     ap=[[256, 128], [128 * 256, 16], [1, 256]])
    nc.scalar.dma_start(out=bd_sb, in_=src_bd)
    nc.scalar.dma_start(out=ident_sb, in_=d_ident)
    nc.scalar.dma_start(out=unitv_sb, in_=d_unitv)
    nc.vector.memset(ones_sb[:].bitcast(F32), 1.0)
    # broadcast tau to all partitions (step-0 partition AP), then reciprocal
    tau_bc = bass.AP(tensor=d_tau.tensor, offset=d_tau.offset, ap=[[0, 128], [1, 1]])
    tau_sb = singles.tile([128, 1], F32, tag="tau", name="tau_sb")
    nc.gpsimd.dma_start(out=tau_sb, in_=tau_bc)
    nc.vector.reciprocal(out=rtau, in_=tau_sb)
    ob_src = bass.AP(tensor=d_ob.tensor, offset=d_ob.offset, ap=[[0, 128], [1, E]])
    nc.gpsimd.dma_start(out=ob_bc, in_=ob_src)
    neg1 = singles.tile([128, 1], F32, tag="neg1", name="neg1")
    nc.vector.memset(neg1, -1.0)

    # ---- attention work units (emitted interleaved with the mask stream) ----
    expT = {}

    def unit_qkvT(fc):
        def go():
            for sp, (s0, sw) in enumerate(SPANS):
                ps = ps_a.tile([128, sw], F32, tag="psa", name="pswork")
                for ec in range(4):
                    nc.tensor.matmul(
                        out=ps,
                        lhsT=qkvwT[ec][:, fc * 128:(fc + 1) * 128],
                        rhs=inputsT[ec][:, s0:s0 + sw],
                        start=(ec == 0), stop=(ec == 3),
                    )
                nc.vector.tensor_copy(out=qkT[fc][:, s0:s0 + sw], in_=ps)
        return go

    def unit_v(mc):
        def go():
            c0, cw = CHUNKS[mc]
            cwp = CWP[mc]
            ps = ps_a.tile([128, E], F32, tag="psa", name="pswork")
            for ec in range(4):
                nc.tensor.matmul(
                    out=ps[:cwp, :],
                    lhsT=inputsT[ec][:, c0:c0 + cwp],
                    rhs=qkvwT[ec][:, 2 * E:3 * E],
                    start=(ec == 0), stop=(ec == 3),
                )
            for h in range(NH):
                nc.vector.tensor_copy(
                    out=v_sb[mc][:cw, h * 65:h * 65 + 64],
                    in_=ps[:cw, h * 64:(h + 1) * 64],
                )
            nc.vector.memset(v_sb[mc][:cw, 64::65], 1.0)
        return go

    def unit_scores(h, mc):
        def go():
            c0, cw = CHUNKS[mc]
            cwp = CWP[mc]
            kt = qkT[4 + h // 2]
            qt = qkT[h // 2]
            hb = 64 * (h % 2)
            et = expool.tile([128, N1P], BF16, tag="expT", name="expT")
            expT[(h, mc)] = et
            ps = ps_big.tile([128, N1P], F32, tag="big", name="ps_sc")
            for sp, (s0, sw) in enumerate(SPANS):
                nc.tensor.matmul(
                    out=ps[:cwp, s0:s0 + sw],
                    lhsT=r32(kt[hb:hb + 64, c0:c0 + cwp]),
                    rhs=r32(qt[hb:hb + 64, s0:s0 + sw]),
                    start=True, stop=True,
                )
            nc.scalar.activation(
                out=et[:cwp, :], in_=ps[:cwp, :],
                func=AFT.Exp, scale=rtau[:cwp],
            )
        return go

    # interleave so scores (ACT exp) work spreads across the whole stream:
    # each quarter emits the two qkvT columns it needs, a v chunk, then the
    # two heads' scores with alternating 0/64 row groups
    units = []
    for q in range(4):
        units.append(unit_qkvT(q))
        units.append(unit_qkvT(4 + q))
        units.append(unit_v(q))
        for mc in range(5):
            units.append(unit_scores(2 * q, mc))
            units.append(unit_scores(2 * q + 1, mc))
    units.append(unit_v(4))

    # ---- is_one computation (psum -> multiplicative mask) ----
    ps_m = [None, None]

    def emit_isone(i):
        tmp = spool.tile([128, NCLS], F32, tag="isotmp", name="isotmp")
        nc.scalar.activation(out=tmp, in_=ps_m[i], func=AFT.Square, bias=neg1)
        nc.scalar.activation(out=isone[i], in_=tmp, func=AFT.Relu, scale=-1.0, bias=1.0)

    def emit_isoT(i):
        # transpose is_one -> isoT (c on partitions); half i fills columns
        # i*128..i*128+128
        for j in range(3):
            cw = 57 if j == 2 else 128
            pst = ps_a.tile([128, 128], BF16, tag="psa", name="pswork_t")
            nc.tensor.transpose(out=pst[:cw, :], in_=isone[i][:, j * 128:j * 128 + cw],
                                identity=ident_sb)
            nc.vector.tensor_copy(out=isoT[j][:cw, i * 128:(i + 1) * 128], in_=pst[:cw, :])

    # ---- the mask stream: 16 fp8 tiles of [128 rows, 2w x 16col x 320c]
    # (1.31MB each). DoubleRow pairs adjacent image columns: each of the 16
    # matmuls per tile ingests 2 columns x 128 rows per cycle; all matmuls of
    # a row-block accumulate the patch sum into ps_m[rt][s, c].
    ui = 0
    for rt in range(2):
        ps_m[rt] = ps_mask.tile([128, NCLS], F32, tag="psmask", name="psmask")
        for T in range(n_tiles):
            t = pre.pop((rt, T), None)
            if t is None:
                t = mask_tile_dma(rt, T)
            for wi in range(2):
                w = 2 * T + wi
                for jp in range(8):
                    nc.tensor.matmul(
                        out=ps_m[rt],
                        lhsT=bd_sb[:, w, :, :],
                        rhs=t[:, wi, jp, :, :NCLS],
                        start=(T == 0 and wi == 0 and jp == 0),
                        stop=(T == n_tiles - 1 and wi == 1 and jp == 7),
                        perf_mode=DR,
                    )
            ti = rt * n_tiles + T
            if ti >= 1:
                budget = 3 if ti < 3 else 4
                for _ in range(budget):
                    if ui < len(units):
                        units[ui]()
                        ui += 1
        emit_isone(rt)
        if rt == 0:
            emit_isoT(0)
    while ui < len(units):
        units[ui]()
        ui += 1
    emit_isoT(1)
    mctx.close()
    ps_out = ctx.enter_context(tc.tile_pool(name="ps_out", bufs=1, space="PSUM"))

    # ---- mask-mult + attn@v with gathered denominators ----
    # Per group of 4 heads: mask-mult expT (color-key side on gpsimd,
    # patch-key side on DVE), attn@v into a 2-bank psum (ones column of v
    # gives the softmax denominator in row 64), evacuate the unnormalized
    # outT to SBUF via the ACT engine, and gather the 4 heads' denominator
    # rows at partitions {0,32,64,96} of a shared psum tile via K=1
    # unit-vector matmuls. Then a single reciprocal per span serves the whole
    # group; the PE broadcasts each head's reciprocal row into a [64, 570]
    # psum and DVE normalizes outT in place reading straight from PSUM.
    outT = [opool.tile([64, N1P], BF16, tag="outT", name="outT") for _ in range(NH)]
    for g in range(2):
        den_ps = {}
        for sp, (s0, sw) in enumerate(SPANS):
            den_ps[sp] = ps_out.tile([128, sw], F32, tag=f"denps{sp}", name="denps", bufs=1)
        for h4 in range(4):
            h = g * 4 + h4
            for mc in range(5):
                c0, cw = CHUNKS[mc]
                et = expT[(h, mc)]
                if mc < 2:
                    nc.gpsimd.tensor_mul(
                        out=et[:cw, SEQ:N1], in0=et[:cw, SEQ:N1], in1=isone[mc])
                else:
                    nc.vector.tensor_mul(
                        out=et[:cw, 0:SEQ], in0=et[:cw, 0:SEQ], in1=isoT[mc - 2][:cw, :])
            rec = spool.tile([65, N1P], F32R, tag="rec", name="rec")
            pso = ps_big.tile([65, N1P], F32, tag="big", name="psout")
            for sp, (s0, sw) in enumerate(SPANS):
                for mc in range(5):
                    c0, cw = CHUNKS[mc]
                    nc.tensor.matmul(
                        out=pso[:, s0:s0 + sw],
                        lhsT=v_sb[mc][:cw, h * 65:(h + 1) * 65],
                        rhs=expT[(h, mc)][:cw, s0:s0 + sw],
                        start=(mc == 0), stop=(mc == 4),
                    )
            with nc.allow_low_precision(reason="f32r copies"):
                nc.scalar.activation(out=rec[64:65, :], in_=pso[64:65, :],
                                     func=AFT.Copy)
                nc.scalar.activation(out=outT[h], in_=pso[0:64, :],
                                     func=AFT.Copy)
            for sp, (s0, sw) in enumerate(SPANS):
                nc.tensor.matmul(
                    out=den_ps[sp],
                    lhsT=r32(unitv_sb[64:65, h4 * 128:(h4 + 1) * 128]),
                    rhs=r32(rec[64:65, s0:s0 + sw]),
                    start=(h4 == 0), stop=(h4 == 3),
                )
        drec = {}
        for sp, (s0, sw) in enumerate(SPANS):
            dr = spool.tile([128, sw], F32R, tag=f"drec{sp}", name=f"drec{sp}")
            with nc.allow_low_precision(reason="f32r reciprocal"):
                nc.vector.reciprocal(out=dr, in_=den_ps[sp])
            drec[sp] = dr
        for h4 in range(4):
            h = g * 4 + h4
            psb = ps_big.tile([64, N1P], F32, tag="big", name="psb")
            for sp, (s0, sw) in enumerate(SPANS):
                nc.tensor.matmul(
                    out=psb[:, s0:s0 + sw],
                    lhsT=r32(ones_sb[32 * h4:32 * h4 + 1, :]),
                    rhs=drec[sp][32 * h4:32 * h4 + 1, :],
                    start=True, stop=True,
                    tile_position=(32 * h4, 0),
                )
            with nc.allow_low_precision(reason="in-place normalize"):
                nc.vector.tensor_mul(out=outT[h], in0=outT[h], in1=psb)

    # ---- o_proj + bias + store ----
    for mc in range(5):
        c0, cw = CHUNKS[mc]
        cwp = CWP[mc]
        psf = ps_a.tile([128, E], F32, tag="psa", name="psf")
        for h in range(NH):
            nc.tensor.matmul(
                out=psf[:cwp, :],
                lhsT=outT[h][:, c0:c0 + cwp],
                rhs=owT[h],
                start=(h == 0), stop=(h == NH - 1),
            )
        fin = spool.tile([128, E], BF16, tag="fin", name="fin")
        nc.vector.tensor_add(out=fin[:cw, :], in0=psf[:cw, :], in1=ob_bc[:cw, :])
        nc.sync.dma_start(out=d_out[c0:c0 + cw, :], in_=fin[:cw, :])

    ctx.close()


def _constants():
    # block-diag: bd[w][r, s'] = 1 iff s' == (r//16)*16 + w; duplicated in
    # pairs for DoubleRow (both elements of a column pair share the map)
    bd = np.zeros((16, 128, 2, 128), dtype=np.float32)
    r = np.arange(128)
    for w in range(16):
        bd[w, r, 0, (r // 16) * 16 + w] = 1.0
        bd[w, r, 1, (r // 16) * 16 + w] = 1.0
    ident = np.eye(128, dtype=ml_dtypes.bfloat16)
    unitv = np.zeros((128, 512), dtype=np.float32)
    for h4 in range(4):
        unitv[:, h4 * 128 + 32 * h4] = 1.0
    return bd.reshape(16, 128, 256).astype(ml_dtypes.float8_e4m3), ident, unitv


def kernel(x, colors, mask, qkv_w, o_w, o_b, tau):
    global LAST_RESULT
    if "nc" not in _CACHED:
        _CACHED["nc"] = _build_program()
    nc = _CACHED["nc"]

    bd, ident, unitv = _constants()
    # pack weight layouts to match SBUF tiles exactly: [part, chunk, col]
    qkv_wT = np.asarray(qkv_w, dtype=np.float32).T.astype(ml_dtypes.bfloat16)
    qkv_wT = np.ascontiguousarray(
        qkv_wT.reshape(4, 128, 3 * E).transpose(1, 0, 2)).reshape(128, 4 * 3 * E)
    o_wT = np.asarray(o_w, dtype=np.float32).T.astype(ml_dtypes.bfloat16)
    o_wT = np.ascontiguousarray(
        o_wT.reshape(8, 64, E).transpose(1, 0, 2)).reshape(64, 8 * E)
    o_b2 = np.asarray(o_b, dtype=np.float32).reshape(1, E)
    tau2 = np.asarray(tau, dtype=np.float32).reshape(1, 1)

    # mask values are exactly 0.0/1.0 -> cast to fp8 is lossless and quarters
    # the HBM stream; pad the c dim to 320 so DoubleRow pair strides are
    # 16B-aligned
    m8 = np.zeros((B, IMG, IMG, NCP), dtype=ml_dtypes.float8_e4m3)
    m8[..., :NCLS] = np.asarray(mask, dtype=np.float32).astype(ml_dtypes.float8_e4m3)

    in_maps = []
    for b in range(B):
        xTf = np.concatenate([np.asarray(x[b]), np.asarray(colors[b])],
                             axis=0).T.astype(ml_dtypes.bfloat16)
        xT = np.zeros((128, 4, N1P), dtype=ml_dtypes.bfloat16)
        xT[:, :, :N1] = xTf.reshape(4, 128, N1).transpose(1, 0, 2)
        xT = xT.reshape(128, 4 * N1P)
        mb = m8[b].reshape(IMG, IMG * NCP)
        in_maps.append({
            "xT": xT, "mask": mb, "qkv_wT": qkv_wT, "o_wT": o_wT,
            "o_b": o_b2, "tau": tau2, "bd": bd, "ident": ident, "unitv": unitv,
        })

    res = run_bass_kernel_spmd(nc, in_maps, list(range(B)))
    LAST_RESULT = res
    out = np.stack([res.results[i]["out"] for i in range(B)]).astype(np.float32)
    return out



# revision 2
# speedup vs baseline: 1.2312x; 1.2312x over previous
"""ColorAttention Trainium2 kernel.

Data-parallel over batch: core b handles batch element b.
Per core:
  - mask [256,256,313] is cast to fp8 (0/1 values, lossless) on the host and
    streamed from HBM (20.9MB with c padded to 320), then patch-reduced via
    block-diagonal ones matmuls on the PE in fp8 DoubleRow mode (2 image
    columns per PE cycle, PSUM accumulation), giving m[s,c] = sum over 16x16
    patch. Multiplicative attention mask is_one(m) = relu(1-(m-1)^2)
    (exact for integer m: 1 iff m==1).
  - attention computed in transposed layout throughout:
      qkvT[f,n] = sum_e qkv_wT[e,f] * inputsT[e,n]
      scoresT[m,n] = sum_d kT[d,m] qT[d,n];  expT = exp(scoresT/tau) * mask
      outT_aug[d|1,n] = sum_m v_aug[m,d|1] expT[m,n]   (row 64 = denom)
      out[n,g] = (sum_{h,d} (outT_h/denom_h)[d,n] o_wT[h*64+d,g]) + o_b
  - all attention matmuls in bf16 (1 cyc/col at any width); heads packed in
    pairs on the 128 partitions for normalize / o_proj.
  - setup DMAs ride the idle SP HWDGE ring (bd first) so the ACT engine is
    free for exp and the mask stream (gpsimd SWDGE ring) is unobstructed.
  - per-head softmax denominators are ACT-copied from psum row 64 straight to
    partitions {0,32,64,96} of a gather tile; one DVE reciprocal per 4 heads;
    PE broadcasts each recip row into the matching 64-partition half of a
    [128,570] psum so one DVE mul normalizes a head pair in place.
"""

import numpy as np
import ml_dtypes

# tolerate environments without the optional NTFF profile hook module when
# BASS_TRACE is set externally
try:
    import antenv.axon_hooks  # noqa: F401
except Exception:
    import sys as _sys
    import types as _types
    _m = _types.ModuleType("antenv.axon_hooks")
    _m.set_axon_ntff_profile_hook = lambda h: None
    _m.get_axon_ntff_profile_hook = lambda: None
    try:
        import antenv
        antenv.axon_hooks = _m
        _sys.modules["antenv.axon_hooks"] = _m
    except Exception:
        pass

import concourse.bass as bass
import concourse.mybir as mybir
import concourse.tile as tile
from concourse import bacc
from concourse.bass_utils import run_bass_kernel_spmd

F32 = mybir.dt.float32
BF16 = mybir.dt.bfloat16
FP8 = mybir.dt.float8e4
AFT = mybir.ActivationFunctionType
DR = mybir.MatmulPerfMode.DoubleRow

B = 8
SEQ = 256
NCLS = 313
NCP = 320  # c dim padded to a 16B multiple so fp8 DoubleRow strides are legal
E = 512
NH = 8
HD = 64
N1 = SEQ + NCLS  # 569
P = 16
IMG = 256

# n/m chunking of the 569 token dim.
N1P = 570
CHUNKS = [(0, 128), (128, 128), (256, 128), (384, 128), (512, 57)]
CWP = [128, 128, 128, 128, 58]
SPANS = [(0, 512), (512, 58)]

# mask stream tiling: (row_block, col0, width). Small lead-in tiles so the
# first PE work starts early; 64-col (2.62MB) tiles once the pipe is primed.
TILES = [
    (0, 0, 16), (0, 16, 16), (0, 32, 32), (0, 64, 64), (0, 128, 64),
    (0, 192, 64),
    (1, 0, 64), (1, 64, 64), (1, 128, 64), (1, 192, 64),
]
# attention work units interleaved after each tile's matmuls (53 total)
BUDGET = [0, 0, 2, 8, 9, 9, 9, 9, 7, 0]

LAST_RESULT = None
_CACHED = {}


def _build_program():
    nc = bacc.Bacc("TRN2", target_bir_lowering=False, debug=False, num_devices=B)

    # ---- DRAM I/O ----
    d_xT = nc.dram_tensor("xT", [128, 4 * N1P], BF16, kind="ExternalInput").ap()
    d_mask = nc.dram_tensor("mask", [IMG, IMG * NCP], FP8, kind="ExternalInput").ap()
    d_qkvwT = nc.dram_tensor("qkv_wT", [128, 4 * 3 * E], BF16, kind="ExternalInput").ap()
    d_owP = nc.dram_tensor("o_wP", [128, 4 * E], BF16, kind="ExternalInput").ap()
    d_ob = nc.dram_tensor("o_b", [1, E], F32, kind="ExternalInput").ap()
    d_tau = nc.dram_tensor("tau", [1, 1], F32, kind="ExternalInput").ap()
    d_bd = nc.dram_tensor("bd", [128, 16 * 256], FP8, kind="ExternalInput").ap()
    d_ident = nc.dram_tensor("ident", [128, 128], BF16, kind="ExternalInput").ap()
    d_out = nc.dram_tensor("out", [N1, E], BF16, kind="ExternalOutput").ap()

    with tile.TileContext(nc) as tc:
        _emit(nc, tc, d_xT, d_mask, d_qkvwT, d_owP, d_ob, d_tau, d_bd, d_ident, d_out)

    nc.compile()
    return nc


def _emit(nc, tc, d_xT, d_mask, d_qkvwT, d_owP, d_ob, d_tau, d_bd, d_ident, d_out):
    from contextlib import ExitStack

    ctx = ExitStack()
    singles = ctx.enter_context(tc.tile_pool(name="singles", bufs=1))
    expool = ctx.enter_context(tc.tile_pool(name="expT", bufs=40))
    opool = ctx.enter_context(tc.tile_pool(name="outTsb", bufs=4))
    spool = ctx.enter_context(tc.tile_pool(name="smalls", bufs=2))
    ps_a = ctx.enter_context(tc.tile_pool(name="ps_a", bufs=2, space="PSUM"))
    ps_big = ctx.enter_context(tc.tile_pool(name="ps_big", bufs=2, space="PSUM"))
    mctx = ExitStack()
    mpool = mctx.enter_context(tc.tile_pool(name="mask_stream", bufs=3))
    ps_mask = mctx.enter_context(tc.tile_pool(name="ps_mask", bufs=2, space="PSUM"))

    # ---- persistent SBUF ----
    inputsT4 = singles.tile([128, 4, N1P], BF16, tag="inT", name="inputsT4")
    inputsT = [inputsT4[:, i, :] for i in range(4)]
    qkvwT4 = singles.tile([128, 4, 3 * E], BF16, tag="qkvwT", name="qkvwT4")
    qkvwT = [qkvwT4[:, i, :] for i in range(4)]
    owP = singles.tile([128, 4, E], BF16, tag="owP", name="owP")
    bd_sb = singles.tile([128, 16, 2, 128], FP8, tag="bd", name="bd_sb")
    ident_sb = singles.tile([128, 128], BF16, tag="ident", name="ident_sb")
    ones_sb = singles.tile([128, 64], BF16, tag="ones", name="ones_sb")
    rtau = singles.tile([128, 1], F32, tag="rtau", name="rtau")
    ob_bc = singles.tile([128, E], F32, tag="ob", name="ob_bc")
    qkT = [singles.tile([128, N1P], BF16, tag=f"qkT{i}", name=f"qkT{i}") for i in range(8)]
    v_sb = [singles.tile([128, NH, HD + 1], BF16, tag=f"vsb{i}", name=f"v_sb{i}") for i in range(5)]
    isone = [singles.tile([128, NCLS], BF16, tag=f"iso{i}", name=f"isone{i}") for i in range(2)]
    isoT = [singles.tile([128, SEQ], BF16, tag=f"isoT{i}", name=f"isoT{i}") for i in range(3)]
    den4 = [singles.tile([128, N1P], BF16, tag=f"den{g}", name=f"den4_{g}") for g in range(2)]
    drec = [singles.tile([128, N1P], BF16, tag=f"drec{g}", name=f"drec{g}") for g in range(2)]

    # ---- short HAM warmup: keep the PE busy while the setup DMAs and the
    # first mask tiles land (the HAM SHORT window needs ~3.4us of activity) ----
    scr = singles.tile([128, 640], BF16, tag="scr", name="scr")
    nc.vector.memset(scr, 1.0)
    ps_warm = ps_a.tile([128, 512], F32, tag="psa", name="ps_warm")
    for _ in range(10):
        nc.tensor.matmul(out=ps_warm, lhsT=scr[:, 0:128], rhs=scr[:, 128:640],
                         start=True, stop=True)

    # ---- mask stream tiles on the SWDGE (gpsimd) ring; the two broadcast
    # loads (tau, o_b need partition-replication -> SWDGE) go first ----
    tau_bc = bass.AP(tensor=d_tau.tensor, offset=d_tau.offset, ap=[[0, 128], [1, 1]])
    tau_sb = singles.tile([128, 1], F32, tag="tau", name="tau_sb")
    nc.gpsimd.dma_start(out=tau_sb, in_=tau_bc)
    ob_src = bass.AP(tensor=d_ob.tensor, offset=d_ob.offset, ap=[[0, 128], [1, E]])
    nc.gpsimd.dma_start(out=ob_bc, in_=ob_src)
    nc.vector.reciprocal(out=rtau, in_=tau_sb)
    nc.vector.memset(ones_sb, 1.0)
    neg1 = singles.tile([128, 1], F32, tag="neg1", name="neg1")
    nc.vector.memset(neg1, -1.0)

    def mask_tile_dma(idx):
        rt, c0, w = TILES[idx]
        t = mpool.tile([128, w // 16, 8, 2, NCP], FP8, tag="mstream", name="mstream")
        src = bass.AP(
            tensor=d_mask.tensor,
            offset=d_mask.offset + rt * 128 * IMG * NCP + c0 * NCP,
            ap=[[IMG * NCP, 128], [1, w * NCP]],
        )
        nc.gpsimd.dma_start(out=t[:, : w // 16], in_=src)
        return t

    pre = {}
    for idx in range(3):
        pre[idx] = mask_tile_dma(idx)

    # ---- setup DMAs on the SP HWDGE ring (idle otherwise): bd first so the
    # mask matmuls are never weight-blocked, then the attention operands ----
    nc.sync.dma_start(out=bd_sb, in_=d_bd)
    nc.sync.dma_start(out=inputsT4, in_=d_xT)
    nc.sync.dma_start(out=qkvwT4, in_=d_qkvwT)
    nc.sync.dma_start(out=owP, in_=d_owP)
    nc.sync.dma_start(out=ident_sb, in_=d_ident)

    # ---- attention work units (emitted interleaved with the mask stream) ----
    expT = {}

    def unit_qkvT(fc):
        def go():
            for s0, sw in SPANS:
                ps = ps_a.tile([128, sw], F32, tag="psa", name="pswork")
                for ec in range(4):
                    nc.tensor.matmul(
                        out=ps,
                        lhsT=qkvwT[ec][:, fc * 128:(fc + 1) * 128],
                        rhs=inputsT[ec][:, s0:s0 + sw],
                        start=(ec == 0), stop=(ec == 3),
                    )
                with nc.allow_low_precision(reason="bf16 qk"):
                    nc.vector.tensor_copy(out=qkT[fc][:, s0:s0 + sw], in_=ps)
        return go

    def unit_v(mc):
        def go():
            c0, cw = CHUNKS[mc]
            cwp = CWP[mc]
            ps = ps_a.tile([128, NH, HD], F32, tag="psa", name="pswork")
            for ec in range(4):
                nc.tensor.matmul(
                    out=ps[:cwp],
                    lhsT=inputsT[ec][:, c0:c0 + cwp],
                    rhs=qkvwT[ec][:, 2 * E:3 * E],
                    start=(ec == 0), stop=(ec == 3),
                )
            with nc.allow_low_precision(reason="bf16 v"):
                nc.vector.tensor_copy(out=v_sb[mc][:cw, :, 0:HD], in_=ps[:cw])
            nc.vector.memset(v_sb[mc][:cw, :, HD:HD + 1], 1.0)
        return go

    def unit_scores(h, mc):
        def go():
            c0, cw = CHUNKS[mc]
            cwp = CWP[mc]
            kt = qkT[4 + h // 2]
            qt = qkT[h // 2]
            hb = 64 * (h % 2)
            et = expool.tile([128, N1P], BF16, tag="expT", name="expT")
            expT[(h, mc)] = et
            ps = ps_big.tile([128, N1P], F32, tag="big", name="ps_sc")
            for s0, sw in SPANS:
                nc.tensor.matmul(
                    out=ps[:cwp, s0:s0 + sw],
                    lhsT=kt[hb:hb + 64, c0:c0 + cwp],
                    rhs=qt[hb:hb + 64, s0:s0 + sw],
                    start=True, stop=True,
                )
            nc.scalar.activation(
                out=et[:cwp, :], in_=ps[:cwp, :],
                func=AFT.Exp, scale=rtau[:cwp],
            )
        return go

    # interleave so scores (ACT exp) work spreads across the whole stream
    units = []
    for q in range(4):
        units.append(unit_qkvT(q))
        units.append(unit_qkvT(4 + q))
        units.append(unit_v(q))
        for mc in range(5):
            units.append(unit_scores(2 * q, mc))
            units.append(unit_scores(2 * q + 1, mc))
    units.append(unit_v(4))

    # ---- is_one computation (psum -> multiplicative mask) ----
    ps_m = [None, None]

    def emit_isone(i):
        tmp = spool.tile([128, NCLS], F32, tag="isotmp", name="isotmp")
        nc.scalar.activation(out=tmp, in_=ps_m[i], func=AFT.Square, bias=neg1)
        nc.scalar.activation(out=isone[i], in_=tmp, func=AFT.Relu, scale=-1.0, bias=1.0)

    def emit_isoT(i):
        # transpose is_one -> isoT (c on partitions); half i fills columns
        # i*128..i*128+128
        for j in range(3):
            cw = 57 if j == 2 else 128
            pst = ps_a.tile([128, 128], BF16, tag="psa", name="pswork_t")
            nc.tensor.transpose(out=pst[:cw, :], in_=isone[i][:, j * 128:j * 128 + cw],
                                identity=ident_sb)
            nc.vector.tensor_copy(out=isoT[j][:cw, i * 128:(i + 1) * 128], in_=pst[:cw, :])

    # ---- the mask stream: fp8 tiles of [128 rows, w cols x 320c].
    # DoubleRow pairs adjacent image columns; all matmuls of a row-block
    # accumulate the patch sum into ps_m[rt][s, c].
    ui = 0
    last_rt = -1
    for idx, (rt, c0, w) in enumerate(TILES):
        if rt != last_rt:
            ps_m[rt] = ps_mask.tile([128, NCLS], F32, tag="psmask", name="psmask")
            last_rt = rt
        t = pre.pop(idx, None)
        if t is None:
            t = mask_tile_dma(idx)
        first_tile = (c0 == 0)
        last_tile = (c0 + w == IMG)
        for wi in range(w // 16):
            wg = c0 // 16 + wi
            for jp in range(8):
                nc.tensor.matmul(
                    out=ps_m[rt],
                    lhsT=bd_sb[:, wg, :, :],
                    rhs=t[:, wi, jp, :, :NCLS],
                    start=(first_tile and wi == 0 and jp == 0),
                    stop=(last_tile and wi == w // 16 - 1 and jp == 7),
                    perf_mode=DR,
                )
        for _ in range(BUDGET[idx]):
            if ui < len(units):
                units[ui]()
                ui += 1
        if last_tile:
            emit_isone(rt)
            emit_isoT(rt)
    while ui < len(units):
        units[ui]()
        ui += 1
    mctx.close()
    ps_nrm = ctx.enter_context(tc.tile_pool(name="ps_nrm", bufs=1, space="PSUM"))

    # ---- mask-mult + attn@v + normalize (head pairs) ----
    # Per head: multiply expT by the mask (color-key side on gpsimd, patch-key
    # side on DVE), attn@v into a 2-bank psum (ones column of v gives the
    # softmax denominator in row 64), evacuate rows 0..63 to the head-pair
    # outP tile via ACT and the denominator row straight to partition 32*h4 of
    # the group's den4 tile. One DVE reciprocal per 4 heads; the PE broadcasts
    # each head's recip row into its 64-partition half of a [128,570] psum and
    # one DVE mul normalizes the pair in place.
    outP = [opool.tile([128, N1P], BF16, tag="outP", name="outP") for _ in range(4)]
    for g in range(2):
        for h4 in range(4):
            h = g * 4 + h4
            for mc in range(5):
                c0, cw = CHUNKS[mc]
                et = expT[(h, mc)]
                if mc == 0:
                    nc.gpsimd.tensor_mul(
                        out=et[:cw, SEQ:N1], in0=et[:cw, SEQ:N1], in1=isone[mc])
                elif mc == 1:
                    nc.vector.tensor_mul(
                        out=et[:cw, SEQ:N1], in0=et[:cw, SEQ:N1], in1=isone[mc])
                else:
                    nc.vector.tensor_mul(
                        out=et[:cw, 0:SEQ], in0=et[:cw, 0:SEQ], in1=isoT[mc - 2][:cw, :])
            pso = ps_big.tile([65, N1P], F32, tag="big", name="psout")
            for s0, sw in SPANS:
                for mc in range(5):
                    c0, cw = CHUNKS[mc]
                    nc.tensor.matmul(
                        out=pso[:, s0:s0 + sw],
                        lhsT=v_sb[mc][:cw, h, :],
                        rhs=expT[(h, mc)][:cw, s0:s0 + sw],
                        start=(mc == 0), stop=(mc == 4),
                    )
            with nc.allow_low_precision(reason="bf16 evac"):
                nc.scalar.activation(
                    out=outP[h // 2][64 * (h % 2):64 * (h % 2) + 64, :],
                    in_=pso[0:64, :], func=AFT.Copy)
                nc.scalar.activation(
                    out=den4[g][32 * h4:32 * h4 + 1, :],
                    in_=pso[64:65, :], func=AFT.Copy)
        with nc.allow_low_precision(reason="bf16 recip"):
            nc.vector.reciprocal(out=drec[g], in_=den4[g])
        for jj in range(2):
            pairidx = 2 * g + jj
            psb = ps_nrm.tile([128, N1P], F32, tag="psb", name="psb")
            for hh in range(2):
                h4 = 2 * jj + hh
                for s0, sw in SPANS:
                    nc.tensor.matmul(
                        out=psb[64 * hh:64 * hh + 64, s0:s0 + sw],
                        lhsT=ones_sb[32 * h4:32 * h4 + 1, :],
                        rhs=drec[g][32 * h4:32 * h4 + 1, s0:s0 + sw],
                        start=True, stop=True,
                        tile_position=(32 * h4, 64 * hh),
                    )
            with nc.allow_low_precision(reason="in-place normalize"):
                nc.vector.tensor_mul(out=outP[pairidx], in0=outP[pairidx], in1=psb)

    # ---- o_proj + bias + store ----
    for mc in range(5):
        c0, cw = CHUNKS[mc]
        cwp = CWP[mc]
        psf = ps_a.tile([128, E], F32, tag="psa", name="psf")
        for j in range(4):
            nc.tensor.matmul(
                out=psf[:cwp, :],
                lhsT=outP[j][:, c0:c0 + cwp],
                rhs=owP[:, j, :],
                start=(j == 0), stop=(j == 3),
            )
        fin = spool.tile([128, E], BF16, tag="fin", name="fin")
        nc.vector.tensor_add(out=fin[:cw, :], in0=psf[:cw, :], in1=ob_bc[:cw, :])
        nc.sync.dma_start(out=d_out[c0:c0 + cw, :], in_=fin[:cw, :])

    ctx.close()


def _constants():
    # block-diag: bd[w][r, s'] = 1 iff s' == (r//16)*16 + w; duplicated in
    # pairs for DoubleRow (both elements of a column pair share the map).
    # Packed host-side as [128 partitions, 16*2*128] so the load is one
    # clean 4KB-per-partition transfer.
    bd = np.zeros((128, 16, 2, 128), dtype=np.float32)
    r = np.arange(128)
    for w in range(16):
        bd[r, w, 0, (r // 16) * 16 + w] = 1.0
        bd[r, w, 1, (r // 16) * 16 + w] = 1.0
    ident = np.eye(128, dtype=ml_dtypes.bfloat16)
    return bd.reshape(128, 16 * 256).astype(ml_dtypes.float8_e4m3), ident


def kernel(x, colors, mask, qkv_w, o_w, o_b, tau):
    global LAST_RESULT
    if "nc" not in _CACHED:
        _CACHED["nc"] = _build_program()
    nc = _CACHED["nc"]

    bd, ident = _constants()
    # pack weight layouts to match SBUF tiles exactly: [part, chunk, col]
    qkv_wT = np.asarray(qkv_w, dtype=np.float32).T.astype(ml_dtypes.bfloat16)
    qkv_wT = np.ascontiguousarray(
        qkv_wT.reshape(4, 128, 3 * E).transpose(1, 0, 2)).reshape(128, 4 * 3 * E)
    # o_w as head-pair blocks: pair j rows 0:64 = head 2j, 64:128 = head 2j+1
    o_wT = np.asarray(o_w, dtype=np.float32).T.astype(ml_dtypes.bfloat16)
    o_wP = np.ascontiguousarray(
        o_wT.reshape(4, 128, E).transpose(1, 0, 2)).reshape(128, 4 * E)
    o_b2 = np.asarray(o_b, dtype=np.float32).reshape(1, E)
    tau2 = np.asarray(tau, dtype=np.float32).reshape(1, 1)

    # mask values are exactly 0.0/1.0 -> cast to fp8 is lossless and quarters
    # the HBM stream; pad the c dim to 320 so DoubleRow pair strides are
    # 16B-aligned
    m8 = np.zeros((B, IMG, IMG, NCP), dtype=ml_dtypes.float8_e4m3)
    m8[..., :NCLS] = np.asarray(mask, dtype=np.float32).astype(ml_dtypes.float8_e4m3)

    in_maps = []
    for b in range(B):
        xTf = np.concatenate([np.asarray(x[b]), np.asarray(colors[b])],
                             axis=0).T.astype(ml_dtypes.bfloat16)
        xT = np.zeros((128, 4, N1P), dtype=ml_dtypes.bfloat16)
        xT[:, :, :N1] = xTf.reshape(4, 128, N1).transpose(1, 0, 2)
        xT = xT.reshape(128, 4 * N1P)
        mb = m8[b].reshape(IMG, IMG * NCP)
        in_maps.append({
            "xT": xT, "mask": mb, "qkv_wT": qkv_wT, "o_wP": o_wP,
            "o_b": o_b2, "tau": tau2, "bd": bd, "ident": ident,
        })

    res = run_bass_kernel_spmd(nc, in_maps, list(range(B)))
    LAST_RESULT = res
    out = np.stack([res.results[i]["out"] for i in range(B)]).astype(np.float32)
    return out


# revision 15
# speedup vs baseline: 1.2756x; 1.0361x over previous
"""ColorAttention Trainium2 kernel.

Data-parallel over batch: core b handles batch element b.
Per core:
  - mask [256,256,313] is cast to fp8 (0/1 values, lossless) on the host and
    streamed from HBM (20.9MB with c padded to 320), then patch-reduced via
    block-diagonal ones matmuls on the PE in fp8 DoubleRow mode (2 image
    columns per PE cycle, PSUM accumulation), giving m[s,c] = sum over 16x16
    patch. Multiplicative attention mask is_one(m) = relu(1-(m-1)^2)
    (exact for integer m: 1 iff m==1).
  - attention computed in transposed layout throughout:
      qkvT[f,n] = sum_e qkv_wT[e,f] * inputsT[e,n]
      scoresT[m,n] = sum_d kT[d,m] qT[d,n];  expT = exp(scoresT/tau) * mask
      outT_aug[d|1,n] = sum_m v_aug[m,d|1] expT[m,n]   (row 64 = denom)
      out[n,g] = (sum_{h,d} (outT_h/denom_h)[d,n] o_wT[h*64+d,g]) + o_b
  - all attention matmuls in bf16 (1 cyc/col at any width); heads packed in
    pairs on the 128 partitions for normalize / o_proj.
  - setup DMAs ride the idle SP HWDGE ring (bd first) so the ACT engine is
    free for exp and the mask stream (gpsimd SWDGE ring) is unobstructed.
  - per-head softmax denominators are ACT-copied from psum row 64 straight to
    partitions {0,32,64,96} of a gather tile; one DVE reciprocal per 4 heads;
    PE broadcasts each recip row into the matching 64-partition half of a
    [128,570] psum so one DVE mul normalizes a head pair in place.
"""

import numpy as np
import ml_dtypes

# tolerate environments without the optional NTFF profile hook module when
# BASS_TRACE is set externally
try:
    import antenv.axon_hooks  # noqa: F401
except Exception:
    import sys as _sys
    import types as _types
    _m = _types.ModuleType("antenv.axon_hooks")
    _m.set_axon_ntff_profile_hook = lambda h: None
    _m.get_axon_ntff_profile_hook = lambda: None
    try:
        import antenv
        antenv.axon_hooks = _m
        _sys.modules["antenv.axon_hooks"] = _m
    except Exception:
        pass

import concourse.bass as bass
import concourse.mybir as mybir
import concourse.tile as tile
from concourse import bacc
from concourse.bass_utils import run_bass_kernel_spmd

F32 = mybir.dt.float32
F32R = mybir.dt.float32r
BF16 = mybir.dt.bfloat16
FP8 = mybir.dt.float8e4
AFT = mybir.ActivationFunctionType
DR = mybir.MatmulPerfMode.DoubleRow

B = 8
SEQ = 256
NCLS = 313
NCP = 320  # c dim padded to a 16B multiple so fp8 DoubleRow strides are legal
E = 512
NH = 8
HD = 64
N1 = SEQ + NCLS  # 569
P = 16
IMG = 256

# n/m chunking of the 569 token dim.
N1P = 570
CHUNKS = [(0, 128), (128, 128), (256, 128), (384, 128), (512, 57)]
CWP = [128, 128, 128, 128, 58]
SPANS = [(0, 512), (512, 58)]

# mask stream tiling: (row_block, col0, width). Small lead-in tiles so the
# first PE work starts early; 64-col (2.62MB) tiles once the pipe is primed.
# The first four tiles ride the sync HWDGE ring (which starves the SWDGE ring
# while active, so urgent data must go there); the rest stream on SWDGE.
TILES = [
    (0, 0, 16), (0, 16, 16), (0, 32, 32), (0, 64, 64), (0, 128, 64),
    (0, 192, 64),
    (1, 0, 64), (1, 64, 64), (1, 128, 64), (1, 192, 64),
]
N_SYNC_TILES = 4
# attention work units interleaved after each tile's matmuls (33 total)
BUDGET = [0, 0, 0, 2, 5, 5, 5, 5, 5, 6]

LAST_RESULT = None
_CACHED = {}


def r32(ap):
    if ap.dtype == F32R:
        return ap
    return ap.bitcast(F32R)


def _build_program():
    nc = bacc.Bacc("TRN2", target_bir_lowering=False, debug=False, num_devices=B)

    # ---- DRAM I/O ----
    d_xT = nc.dram_tensor("xT", [128, 4 * N1P], BF16, kind="ExternalInput").ap()
    d_mask = nc.dram_tensor("mask", [IMG, IMG * NCP], FP8, kind="ExternalInput").ap()
    d_qkvwT = nc.dram_tensor("qkv_wT", [128, 4 * 3 * E], BF16, kind="ExternalInput").ap()
    d_owP = nc.dram_tensor("o_wP", [128, 4 * E], BF16, kind="ExternalInput").ap()
    d_ob = nc.dram_tensor("o_b", [1, E], F32, kind="ExternalInput").ap()
    d_tau = nc.dram_tensor("tau", [1, 1], F32, kind="ExternalInput").ap()
    d_bd = nc.dram_tensor("bd", [128, 16 * 256], FP8, kind="ExternalInput").ap()
    d_ident = nc.dram_tensor("ident", [128, 128], BF16, kind="ExternalInput").ap()
    d_out = nc.dram_tensor("out", [N1, E], BF16, kind="ExternalOutput").ap()

    with tile.TileContext(nc) as tc:
        _emit(nc, tc, d_xT, d_mask, d_qkvwT, d_owP, d_ob, d_tau, d_bd, d_ident, d_out)

    nc.compile()
    return nc


def _emit(nc, tc, d_xT, d_mask, d_qkvwT, d_owP, d_ob, d_tau, d_bd, d_ident, d_out):
    from contextlib import ExitStack

    ctx = ExitStack()
    singles = ctx.enter_context(tc.tile_pool(name="singles", bufs=1))
    expool = ctx.enter_context(tc.tile_pool(name="expT", bufs=40))
    opool = ctx.enter_context(tc.tile_pool(name="outTsb", bufs=4))
    spool = ctx.enter_context(tc.tile_pool(name="smalls", bufs=2))
    ps_a = ctx.enter_context(tc.tile_pool(name="ps_a", bufs=2, space="PSUM"))
    ps_big = ctx.enter_context(tc.tile_pool(name="ps_big", bufs=2, space="PSUM"))
    mctx = ExitStack()
    mh_small = mctx.enter_context(tc.tile_pool(name="mh_small", bufs=2))
    mh_mid = mctx.enter_context(tc.tile_pool(name="mh_mid", bufs=1))
    mpool = mctx.enter_context(tc.tile_pool(name="mask_stream", bufs=3))
    ps_mask = mctx.enter_context(tc.tile_pool(name="ps_mask", bufs=2, space="PSUM"))

    # ---- persistent SBUF ----
    inputsT4 = singles.tile([128, 4, N1P], BF16, tag="inT", name="inputsT4")
    inputsT = [inputsT4[:, i, :] for i in range(4)]
    qkvwT4 = singles.tile([128, 4, 3 * E], BF16, tag="qkvwT", name="qkvwT4")
    qkvwT = [qkvwT4[:, i, :] for i in range(4)]
    owP = singles.tile([128, 4, E], BF16, tag="owP", name="owP")
    bd_sb = singles.tile([128, 16, 2, 128], FP8, tag="bd", name="bd_sb")
    ident_sb = singles.tile([128, 128], BF16, tag="ident", name="ident_sb")
    ones_sb = singles.tile([128, 64], BF16, tag="ones", name="ones_sb")
    rtau = singles.tile([128, 1], F32, tag="rtau", name="rtau")
    ob_bc = singles.tile([128, E], F32, tag="ob", name="ob_bc")
    qkT = [singles.tile([128, N1P], BF16, tag=f"qkT{i}", name=f"qkT{i}") for i in range(8)]
    v_sb = [singles.tile([128, NH, HD + 1], BF16, tag=f"vsb{i}", name=f"v_sb{i}") for i in range(5)]
    isone = [singles.tile([128, NCLS], BF16, tag=f"iso{i}", name=f"isone{i}") for i in range(2)]
    isoT = [singles.tile([128, SEQ], BF16, tag=f"isoT{i}", name=f"isoT{i}") for i in range(3)]
    den4 = [singles.tile([128, N1P], F32, tag=f"den{g}", name=f"den4_{g}") for g in range(2)]
    drec_f = singles.tile([128, N1P], F32, tag="drecf", name="drec_f")
    drec = [singles.tile([128, N1P], BF16, tag=f"drec{g}", name=f"drec{g}") for g in range(2)]

    # ---- short HAM warmup: keep the PE busy while the setup DMAs and the
    # first mask tiles land (the HAM SHORT window needs ~3.4us of activity) ----
    scr = singles.tile([128, 640], BF16, tag="scr", name="scr")
    nc.vector.memset(scr, 1.0)
    ps_warm = ps_a.tile([128, 512], F32, tag="psa", name="ps_warm")
    for _ in range(6):
        nc.tensor.matmul(out=ps_warm, lhsT=scr[:, 0:128], rhs=scr[:, 128:640],
                         start=True, stop=True)

    # ---- the two broadcast loads (tau, o_b need partition-replication ->
    # SWDGE) lead the gpsimd ring ----
    tau_bc = bass.AP(tensor=d_tau.tensor, offset=d_tau.offset, ap=[[0, 128], [1, 1]])
    tau_sb = singles.tile([128, 1], F32, tag="tau", name="tau_sb")
    nc.gpsimd.dma_start(out=tau_sb, in_=tau_bc)
    ob_src = bass.AP(tensor=d_ob.tensor, offset=d_ob.offset, ap=[[0, 128], [1, E]])
    nc.gpsimd.dma_start(out=ob_bc, in_=ob_src)
    nc.vector.reciprocal(out=rtau, in_=tau_sb)
    nc.vector.memset(ones_sb, 1.0)
    neg1 = singles.tile([128, 1], F32, tag="neg1", name="neg1")
    nc.vector.memset(neg1, -1.0)

    def mask_tile_dma(idx):
        rt, c0, w = TILES[idx]
        if w == 16:
            t = mh_small.tile([128, 1, 8, 2, NCP], FP8, tag="mh_s", name="mh_s")
        elif w == 32:
            t = mh_mid.tile([128, 2, 8, 2, NCP], FP8, tag="mh_m", name="mh_m")
        else:
            t = mpool.tile([128, 4, 8, 2, NCP], FP8, tag="mstream", name="mstream")
        src = bass.AP(
            tensor=d_mask.tensor,
            offset=d_mask.offset + rt * 128 * IMG * NCP + c0 * NCP,
            ap=[[IMG * NCP, 128], [1, w * NCP]],
        )
        eng = nc.sync if idx < N_SYNC_TILES else nc.gpsimd
        eng.dma_start(out=t[:, : w // 16], in_=src)
        return t

    # ---- DMA issue order. Sync HWDGE ring (starves SWDGE while active, so
    # it carries everything needed early, most-urgent first): bd -> first two
    # small mask tiles -> xT -> mid/big mask tiles -> qkv weights -> o_proj
    # weights. The SWDGE ring pre-queues the big steady-state tiles; it gets
    # full bandwidth once the sync ring drains. ----
    pre = {}
    nc.sync.dma_start(out=bd_sb, in_=d_bd)
    pre[0] = mask_tile_dma(0)
    pre[1] = mask_tile_dma(1)
    nc.sync.dma_start(out=inputsT4, in_=d_xT)
    pre[2] = mask_tile_dma(2)
    pre[3] = mask_tile_dma(3)
    nc.sync.dma_start(out=qkvwT4, in_=d_qkvwT)
    nc.sync.dma_start(out=owP, in_=d_owP)
    nc.sync.dma_start(out=ident_sb, in_=d_ident)
    pre[4] = mask_tile_dma(4)
    pre[5] = mask_tile_dma(5)

    # ---- attention work units (emitted interleaved with the mask stream) ----
    expT = {}

    def unit_qkvT(fc):
        def go():
            for s0, sw in SPANS:
                ps = ps_a.tile([128, sw], F32, tag="psa", name="pswork")
                for ec in range(4):
                    nc.tensor.matmul(
                        out=ps,
                        lhsT=qkvwT[ec][:, fc * 128:(fc + 1) * 128],
                        rhs=inputsT[ec][:, s0:s0 + sw],
                        start=(ec == 0), stop=(ec == 3),
                    )
                with nc.allow_low_precision(reason="bf16 qk"):
                    nc.vector.tensor_copy(out=qkT[fc][:, s0:s0 + sw], in_=ps)
        return go

    def unit_v(mc):
        def go():
            c0, cw = CHUNKS[mc]
            cwp = CWP[mc]
            ps = ps_a.tile([128, NH, HD], F32, tag="psa", name="pswork")
            for ec in range(4):
                nc.tensor.matmul(
                    out=ps[:cwp],
                    lhsT=inputsT[ec][:, c0:c0 + cwp],
                    rhs=qkvwT[ec][:, 2 * E:3 * E],
                    start=(ec == 0), stop=(ec == 3),
                )
            with nc.allow_low_precision(reason="bf16 v"):
                nc.vector.tensor_copy(out=v_sb[mc][:cw, :, 0:HD], in_=ps[:cw])
            nc.vector.memset(v_sb[mc][:cw, :, HD:HD + 1], 1.0)
        return go

    def unit_scores_pair(q, mc):
        # heads 2q (PE rows 0:64) and 2q+1 (rows 64:128): the two matmuls of a
        # span are emitted adjacently so their disjoint row groups run
        # concurrently on the array.
        def go():
            c0, cw = CHUNKS[mc]
            cwp = CWP[mc]
            kt = qkT[4 + q]
            qt = qkT[q]
            ets = []
            pss = []
            for hh in range(2):
                et = expool.tile([128, N1P], BF16, tag="expT", name="expT")
                expT[(2 * q + hh, mc)] = et
                ets.append(et)
                pss.append(ps_big.tile([128, N1P], F32, tag="big", name="ps_sc"))
            for s0, sw in SPANS:
                for hh in range(2):
                    hb = 64 * hh
                    nc.tensor.matmul(
                        out=pss[hh][:cwp, s0:s0 + sw],
                        lhsT=kt[hb:hb + 64, c0:c0 + cwp],
                        rhs=qt[hb:hb + 64, s0:s0 + sw],
                        start=True, stop=True,
                    )
            for hh in range(2):
                nc.scalar.activation(
                    out=ets[hh][:cwp, :], in_=pss[hh][:cwp, :],
                    func=AFT.Exp, scale=rtau[:cwp],
                )
        return go

    # interleave so scores (ACT exp) work spreads across the whole stream
    units = []
    for q in range(4):
        units.append(unit_qkvT(q))
        units.append(unit_qkvT(4 + q))
        units.append(unit_v(q))
        for mc in range(5):
            units.append(unit_scores_pair(q, mc))
    units.append(unit_v(4))

    # ---- is_one computation (psum -> multiplicative mask) ----
    ps_m = [None, None]

    def emit_isone(i):
        tmp = spool.tile([128, NCLS], F32, tag="isotmp", name="isotmp")
        nc.scalar.activation(out=tmp, in_=ps_m[i], func=AFT.Square, bias=neg1)
        nc.scalar.activation(out=isone[i], in_=tmp, func=AFT.Relu, scale=-1.0, bias=1.0)

    def emit_isoT(i):
        # transpose is_one -> isoT (c on partitions); half i fills columns
        # i*128..i*128+128
        for j in range(3):
            cw = 57 if j == 2 else 128
            pst = ps_a.tile([128, 128], BF16, tag="psa", name="pswork_t")
            nc.tensor.transpose(out=pst[:cw, :], in_=isone[i][:, j * 128:j * 128 + cw],
                                identity=ident_sb)
            nc.vector.tensor_copy(out=isoT[j][:cw, i * 128:(i + 1) * 128], in_=pst[:cw, :])

    # ---- the mask stream: fp8 tiles of [128 rows, w cols x 320c].
    # DoubleRow pairs adjacent image columns; all matmuls of a row-block
    # accumulate the patch sum into ps_m[rt][s, c].
    ui = 0
    last_rt = -1
    for idx, (rt, c0, w) in enumerate(TILES):
        if rt != last_rt:
            ps_m[rt] = ps_mask.tile([128, NCLS], F32, tag="psmask", name="psmask")
            last_rt = rt
        t = pre.pop(idx, None)
        if t is None:
            t = mask_tile_dma(idx)
        first_tile = (c0 == 0)
        last_tile = (c0 + w == IMG)
        for wi in range(w // 16):
            wg = c0 // 16 + wi
            for jp in range(8):
                nc.tensor.matmul(
                    out=ps_m[rt],
                    lhsT=bd_sb[:, wg, :, :],
                    rhs=t[:, wi, jp, :, :NCLS],
                    start=(first_tile and wi == 0 and jp == 0),
                    stop=(last_tile and wi == w // 16 - 1 and jp == 7),
                    perf_mode=DR,
                )
        if last_tile:
            # is_one first so its ACT ops aren't queued behind the units' exps
            emit_isone(rt)
        for _ in range(BUDGET[idx]):
            if ui < len(units):
                units[ui]()
                ui += 1
        if last_tile:
            emit_isoT(rt)
    while ui < len(units):
        units[ui]()
        ui += 1
    mctx.close()
    ps_nrm = ctx.enter_context(tc.tile_pool(name="ps_nrm", bufs=1, space="PSUM"))

    # ---- mask-mult + attn@v + normalize (head pairs) ----
    # Per head: multiply expT by the mask (color-key side on gpsimd, patch-key
    # side on DVE), attn@v into a 2-bank psum (ones column of v gives the
    # softmax denominator in row 64), evacuate rows 0..63 to the head-pair
    # outP tile via ACT and the denominator row straight to partition 32*h4 of
    # the group's den4 tile. One DVE reciprocal per 4 heads; the PE broadcasts
    # each head's recip row into its 64-partition half of a [128,570] psum and
    # one DVE mul normalizes the pair in place.
    outP = [opool.tile([128, N1P], BF16, tag="outP", name="outP") for _ in range(4)]
    for g in range(2):
        for h4 in range(4):
            h = g * 4 + h4
            for mc in range(5):
                c0, cw = CHUNKS[mc]
                et = expT[(h, mc)]
                if mc == 0:
                    nc.gpsimd.tensor_mul(
                        out=et[:cw, SEQ:N1], in0=et[:cw, SEQ:N1], in1=isone[mc])
                elif mc == 1:
                    nc.vector.tensor_mul(
                        out=et[:cw, SEQ:N1], in0=et[:cw, SEQ:N1], in1=isone[mc])
                else:
                    nc.vector.tensor_mul(
                        out=et[:cw, 0:SEQ], in0=et[:cw, 0:SEQ], in1=isoT[mc - 2][:cw, :])
            pso = ps_big.tile([65, N1P], F32, tag="big", name="psout")
            for s0, sw in SPANS:
                for mc in range(5):
                    c0, cw = CHUNKS[mc]
                    nc.tensor.matmul(
                        out=pso[:, s0:s0 + sw],
                        lhsT=v_sb[mc][:cw, h, :],
                        rhs=expT[(h, mc)][:cw, s0:s0 + sw],
                        start=(mc == 0), stop=(mc == 4),
                    )
            with nc.allow_low_precision(reason="bf16 evac"):
                nc.scalar.activation(
                    out=outP[h // 2][64 * (h % 2):64 * (h % 2) + 64, :],
                    in_=pso[0:64, :], func=AFT.Copy)
            nc.scalar.activation(
                out=den4[g][32 * h4:32 * h4 + 1, :],
                in_=pso[64:65, :], func=AFT.Copy)
        nc.vector.reciprocal_approx_fast(out=drec_f, in_=den4[g])
        with nc.allow_low_precision(reason="bf16 recip"):
            nc.vector.tensor_copy(out=drec[g], in_=drec_f)
        for jj in range(2):
            pairidx = 2 * g + jj
            psb = ps_nrm.tile([128, N1P], F32, tag="psb", name="psb")
            # the pair's two row groups (32*h4 vs 32*h4+32) are disjoint, so
            # emitting the two heads' matmuls adjacently per span runs them
            # concurrently on the array
            for s0, sw in SPANS:
                for hh in range(2):
                    h4 = 2 * jj + hh
                    nc.tensor.matmul(
                        out=psb[64 * hh:64 * hh + 64, s0:s0 + sw],
                        lhsT=ones_sb[32 * h4:32 * h4 + 1, :],
                        rhs=drec[g][32 * h4:32 * h4 + 1, s0:s0 + sw],
                        start=True, stop=True,
                        tile_position=(32 * h4, 64 * hh),
                    )
            with nc.allow_low_precision(reason="in-place normalize"):
                nc.vector.tensor_mul(out=outP[pairidx], in0=outP[pairidx], in1=psb)

    # ---- o_proj + bias + store ----
    for mc in range(5):
        c0, cw = CHUNKS[mc]
        cwp = CWP[mc]
        psf = ps_a.tile([128, E], F32, tag="psa", name="psf")
        for j in range(4):
            nc.tensor.matmul(
                out=psf[:cwp, :],
                lhsT=outP[j][:, c0:c0 + cwp],
                rhs=owP[:, j, :],
                start=(j == 0), stop=(j == 3),
            )
        fin = spool.tile([128, E], BF16, tag="fin", name="fin")
        nc.vector.tensor_add(out=fin[:cw, :], in0=psf[:cw, :], in1=ob_bc[:cw, :])
        nc.sync.dma_start(out=d_out[c0:c0 + cw, :], in_=fin[:cw, :])

    ctx.close()


def _constants():
    # block-diag: bd[w][r, s'] = 1 iff s' == (r//16)*16 + w; duplicated in
    # pairs for DoubleRow (both elements of a column pair share the map).
    # Packed host-side as [128 partitions, 16*2*128] so the load is one
    # clean 4KB-per-partition transfer.
    bd = np.zeros((128, 16, 2, 128), dtype=np.float32)
    r = np.arange(128)
    for w in range(16):
        bd[r, w, 0, (r // 16) * 16 + w] = 1.0
        bd[r, w, 1, (r // 16) * 16 + w] = 1.0
    ident = np.eye(128, dtype=ml_dtypes.bfloat16)
    return bd.reshape(128, 16 * 256).astype(ml_dtypes.float8_e4m3), ident


def kernel(x, colors, mask, qkv_w, o_w, o_b, tau):
    global LAST_RESULT
    if "nc" not in _CACHED:
        _CACHED["nc"] = _build_program()
    nc = _CACHED["nc"]

    bd, ident = _constants()
    # pack weight layouts to match SBUF tiles exactly: [part, chunk, col]
    qkv_wT = np.asarray(qkv_w, dtype=np.float32).T.astype(ml_dtypes.bfloat16)
    qkv_wT = np.ascontiguousarray(
        qkv_wT.reshape(4, 128, 3 * E).transpose(1, 0, 2)).reshape(128, 4 * 3 * E)
    # o_w as head-pair blocks: pair j rows 0:64 = head 2j, 64:128 = head 2j+1
    o_wT = np.asarray(o_w, dtype=np.float32).T.astype(ml_dtypes.bfloat16)
    o_wP = np.ascontiguousarray(
        o_wT.reshape(4, 128, E).transpose(1, 0, 2)).reshape(128, 4 * E)
    o_b2 = np.asarray(o_b, dtype=np.float32).reshape(1, E)
    tau2 = np.asarray(tau, dtype=np.float32).reshape(1, 1)

    # mask values are exactly 0.0/1.0 -> cast to fp8 is lossless and quarters
    # the HBM stream; pad the c dim to 320 so DoubleRow pair strides are
    # 16B-aligned
    m8 = np.zeros((B, IMG, IMG, NCP), dtype=ml_dtypes.float8_e4m3)
    m8[..., :NCLS] = np.asarray(mask, dtype=np.float32).astype(ml_dtypes.float8_e4m3)

    in_maps = []
    for b in range(B):
        xTf = np.concatenate([np.asarray(x[b]), np.asarray(colors[b])],
                             axis=0).T.astype(ml_dtypes.bfloat16)
        xT = np.zeros((128, 4, N1P), dtype=ml_dtypes.bfloat16)
        xT[:, :, :N1] = xTf.reshape(4, 128, N1).transpose(1, 0, 2)
        xT = xT.reshape(128, 4 * N1P)
        mb = m8[b].reshape(IMG, IMG * NCP)
        in_maps.append({
            "xT": xT, "mask": mb, "qkv_wT": qkv_wT, "o_wP": o_wP,
            "o_b": o_b2, "tau": tau2, "bd": bd, "ident": ident,
        })

    res = run_bass_kernel_spmd(nc, in_maps, list(range(B)))
    LAST_RESULT = res
    out = np.stack([res.results[i]["out"] for i in range(B)]).astype(np.float32)
    return out


# revision 19
# speedup vs baseline: 1.3378x; 1.0487x over previous
"""ColorAttention Trainium2 kernel.

Data-parallel over batch: core b handles batch element b.
Per core:
  - mask [256,256,313] is cast to fp8 (0/1 values, lossless) on the host and
    streamed from HBM (20.9MB with c padded to 320), then patch-reduced via
    block-diagonal ones matmuls on the PE in fp8 DoubleRow mode (2 image
    columns per PE cycle, PSUM accumulation), giving m[s,c] = sum over 16x16
    patch. Multiplicative attention mask is_one(m) = relu(1-(m-1)^2)
    (exact for integer m: 1 iff m==1).
  - attention computed in transposed layout throughout:
      qkvT[f,n] = sum_e qkv_wT[e,f] * inputsT[e,n]
      scoresT[m,n] = sum_d kT[d,m] qT[d,n];  expT = exp(scoresT/tau) * mask
      outT_aug[d|1,n] = sum_m v_aug[m,d|1] expT[m,n]   (row 64 = denom)
      out[n,g] = (sum_{h,d} (outT_h/denom_h)[d,n] o_wT[h*64+d,g]) + o_b
  - all attention matmuls in bf16 (1 cyc/col at any width); heads packed in
    pairs on the 128 partitions for normalize / o_proj.
  - setup DMAs ride the idle SP HWDGE ring (bd first) so the ACT engine is
    free for exp and the mask stream (gpsimd SWDGE ring) is unobstructed.
  - per-head softmax denominators are ACT-copied from psum row 64 straight to
    partitions {0,32,64,96} of a gather tile; one DVE reciprocal per 4 heads;
    PE broadcasts each recip row into the matching 64-partition half of a
    [128,570] psum so one DVE mul normalizes a head pair in place.
"""

import numpy as np
import ml_dtypes

# tolerate environments without the optional NTFF profile hook module when
# BASS_TRACE is set externally
try:
    import antenv.axon_hooks  # noqa: F401
except Exception:
    import sys as _sys
    import types as _types
    _m = _types.ModuleType("antenv.axon_hooks")
    _m.set_axon_ntff_profile_hook = lambda h: None
    _m.get_axon_ntff_profile_hook = lambda: None
    try:
        import antenv
        antenv.axon_hooks = _m
        _sys.modules["antenv.axon_hooks"] = _m
    except Exception:
        pass

import concourse.bass as bass
import concourse.mybir as mybir
import concourse.tile as tile
from concourse import bacc
from concourse.bass_utils import run_bass_kernel_spmd

F32 = mybir.dt.float32
F32R = mybir.dt.float32r
BF16 = mybir.dt.bfloat16
FP8 = mybir.dt.float8e4
AFT = mybir.ActivationFunctionType
DR = mybir.MatmulPerfMode.DoubleRow

B = 8
SEQ = 256
NCLS = 313
NCP = 320  # c dim padded to a 16B multiple so fp8 DoubleRow strides are legal
E = 512
NH = 8
HD = 64
N1 = SEQ + NCLS  # 569
P = 16
IMG = 256

# n/m chunking of the 569 token dim.
N1P = 570
CHUNKS = [(0, 128), (128, 128), (256, 128), (384, 128), (512, 57)]
CWP = [128, 128, 128, 128, 58]
SPANS = [(0, 512), (512, 58)]

# mask stream tiling: (row_block, col0, width). Small lead-in tiles so the
# first PE work starts early; 64-col (2.62MB) tiles once the pipe is primed.
# The first four tiles ride the sync HWDGE ring (which starves the SWDGE ring
# while active, so urgent data must go there); the rest stream on SWDGE.
TILES = [
    (0, 0, 16), (0, 16, 16), (0, 32, 32), (0, 64, 64), (0, 128, 64),
    (0, 192, 64),
    (1, 0, 64), (1, 64, 64), (1, 128, 64), (1, 192, 64),
]
N_SYNC_TILES = 6
# attention work units interleaved after each tile's matmuls (33 total)
BUDGET = [0, 0, 2, 3, 5, 5, 5, 5, 5, 3]
# scr-based keep-warm matmuls after early tiles (no data deps)
FILLER = [2, 3, 0, 0, 0, 0, 0, 0, 0, 0]

LAST_RESULT = None
_CACHED = {}


def r32(ap):
    if ap.dtype == F32R:
        return ap
    return ap.bitcast(F32R)


def _build_program():
    nc = bacc.Bacc("TRN2", target_bir_lowering=False, debug=False, num_devices=B)

    # ---- DRAM I/O ----
    d_xT = nc.dram_tensor("xT", [128, 4 * N1P], BF16, kind="ExternalInput").ap()
    d_mask = nc.dram_tensor("mask", [IMG, IMG * NCP], FP8, kind="ExternalInput").ap()
    d_qkvwT = nc.dram_tensor("qkv_wT", [128, 4 * 3 * E], BF16, kind="ExternalInput").ap()
    d_owP = nc.dram_tensor("o_wP", [128, 4 * E], BF16, kind="ExternalInput").ap()
    d_ob = nc.dram_tensor("o_b", [1, E], F32, kind="ExternalInput").ap()
    d_tau = nc.dram_tensor("tau", [1, 1], F32, kind="ExternalInput").ap()
    d_bd = nc.dram_tensor("bd", [128, 16 * 256], FP8, kind="ExternalInput").ap()
    d_ident = nc.dram_tensor("ident", [128, 128], BF16, kind="ExternalInput").ap()
    d_out = nc.dram_tensor("out", [N1, E], BF16, kind="ExternalOutput").ap()

    with tile.TileContext(nc) as tc:
        _emit(nc, tc, d_xT, d_mask, d_qkvwT, d_owP, d_ob, d_tau, d_bd, d_ident, d_out)

    nc.compile()
    return nc


def _emit(nc, tc, d_xT, d_mask, d_qkvwT, d_owP, d_ob, d_tau, d_bd, d_ident, d_out):
    from contextlib import ExitStack

    ctx = ExitStack()
    singles = ctx.enter_context(tc.tile_pool(name="singles", bufs=1))
    expool = ctx.enter_context(tc.tile_pool(name="expT", bufs=40))
    opool = ctx.enter_context(tc.tile_pool(name="outTsb", bufs=4))
    spool = ctx.enter_context(tc.tile_pool(name="smalls", bufs=2))
    ps_a = ctx.enter_context(tc.tile_pool(name="ps_a", bufs=2, space="PSUM"))
    ps_big = ctx.enter_context(tc.tile_pool(name="ps_big", bufs=2, space="PSUM"))
    mctx = ExitStack()
    mh_small = mctx.enter_context(tc.tile_pool(name="mh_small", bufs=2))
    mh_mid = mctx.enter_context(tc.tile_pool(name="mh_mid", bufs=1))
    mpool = mctx.enter_context(tc.tile_pool(name="mask_stream", bufs=3))
    ps_mask = mctx.enter_context(tc.tile_pool(name="ps_mask", bufs=2, space="PSUM"))

    # ---- persistent SBUF ----
    inputsT4 = singles.tile([128, 4, N1P], BF16, tag="inT", name="inputsT4")
    inputsT = [inputsT4[:, i, :] for i in range(4)]
    qkvwT4 = singles.tile([128, 4, 3 * E], BF16, tag="qkvwT", name="qkvwT4")
    qkvwT = [qkvwT4[:, i, :] for i in range(4)]
    owP = singles.tile([128, 4, E], BF16, tag="owP", name="owP")
    bd_sb = singles.tile([128, 16, 2, 128], FP8, tag="bd", name="bd_sb")
    ident_sb = singles.tile([128, 128], BF16, tag="ident", name="ident_sb")
    ones_sb = singles.tile([128, 64], BF16, tag="ones", name="ones_sb")
    rtau = singles.tile([128, 1], F32, tag="rtau", name="rtau")
    ob_bc = singles.tile([128, E], F32, tag="ob", name="ob_bc")
    qkT = [singles.tile([128, N1P], BF16, tag=f"qkT{i}", name=f"qkT{i}") for i in range(8)]
    v_sb = [singles.tile([128, NH, HD + 1], BF16, tag=f"vsb{i}", name=f"v_sb{i}") for i in range(5)]
    isone = [singles.tile([128, NCLS], BF16, tag=f"iso{i}", name=f"isone{i}") for i in range(2)]
    isoT = [singles.tile([128, SEQ], BF16, tag=f"isoT{i}", name=f"isoT{i}") for i in range(3)]
    den4 = [singles.tile([128, N1P], F32, tag=f"den{g}", name=f"den4_{g}") for g in range(2)]
    drec_f = singles.tile([128, N1P], F32, tag="drecf", name="drec_f")
    drec = [singles.tile([128, N1P], BF16, tag=f"drec{g}", name=f"drec{g}") for g in range(2)]

    # ---- short HAM warmup: keep the PE busy while the setup DMAs and the
    # first mask tiles land (the HAM SHORT window needs ~3.4us of activity) ----
    scr = singles.tile([128, 640], BF16, tag="scr", name="scr")
    nc.vector.memset(scr, 1.0)
    ps_warm = ps_a.tile([128, 512], F32, tag="psa", name="ps_warm")
    for _ in range(6):
        nc.tensor.matmul(out=ps_warm, lhsT=scr[:, 0:128], rhs=scr[:, 128:640],
                         start=True, stop=True)

    # ---- the two broadcast loads (tau, o_b need partition-replication ->
    # SWDGE) lead the gpsimd ring ----
    tau_bc = bass.AP(tensor=d_tau.tensor, offset=d_tau.offset, ap=[[0, 128], [1, 1]])
    tau_sb = singles.tile([128, 1], F32, tag="tau", name="tau_sb")
    nc.gpsimd.dma_start(out=tau_sb, in_=tau_bc)
    ob_src = bass.AP(tensor=d_ob.tensor, offset=d_ob.offset, ap=[[0, 128], [1, E]])
    nc.gpsimd.dma_start(out=ob_bc, in_=ob_src)
    nc.vector.reciprocal(out=rtau, in_=tau_sb)
    nc.vector.memset(ones_sb, 1.0)
    neg1 = singles.tile([128, 1], F32, tag="neg1", name="neg1")
    nc.vector.memset(neg1, -1.0)

    def mask_tile_dma(idx):
        rt, c0, w = TILES[idx]
        if w == 16:
            t = mh_small.tile([128, 1, 8, 2, NCP], FP8, tag="mh_s", name="mh_s")
        elif w == 32:
            t = mh_mid.tile([128, 2, 8, 2, NCP], FP8, tag="mh_m", name="mh_m")
        else:
            t = mpool.tile([128, 4, 8, 2, NCP], FP8, tag="mstream", name="mstream")
        src = bass.AP(
            tensor=d_mask.tensor,
            offset=d_mask.offset + rt * 128 * IMG * NCP + c0 * NCP,
            ap=[[IMG * NCP, 128], [1, w * NCP]],
        )
        eng = nc.sync if idx < N_SYNC_TILES else nc.gpsimd
        eng.dma_start(out=t[:, : w // 16], in_=src)
        return t

    # ---- DMA issue order. The two rings share SDMA bandwidth roughly
    # fairly, so everything needed early rides the sync HWDGE ring in exact
    # consumption order; the SWDGE ring carries only the steady-state tail
    # tiles, whose issue is naturally delayed by mask-pool slot recycling.
    pre = {}
    nc.sync.dma_start(out=bd_sb, in_=d_bd)
    pre[0] = mask_tile_dma(0)
    pre[1] = mask_tile_dma(1)
    nc.sync.dma_start(out=inputsT4, in_=d_xT)
    nc.sync.dma_start(out=qkvwT4, in_=d_qkvwT)
    pre[2] = mask_tile_dma(2)
    pre[3] = mask_tile_dma(3)
    nc.sync.dma_start(out=owP, in_=d_owP)
    nc.sync.dma_start(out=ident_sb, in_=d_ident)
    pre[4] = mask_tile_dma(4)
    pre[5] = mask_tile_dma(5)

    # ---- attention work units (emitted interleaved with the mask stream) ----
    expT = {}

    def unit_qkvT(fc):
        def go():
            for s0, sw in SPANS:
                ps = ps_a.tile([128, sw], F32, tag="psa", name="pswork")
                for ec in range(4):
                    nc.tensor.matmul(
                        out=ps,
                        lhsT=qkvwT[ec][:, fc * 128:(fc + 1) * 128],
                        rhs=inputsT[ec][:, s0:s0 + sw],
                        start=(ec == 0), stop=(ec == 3),
                    )
                with nc.allow_low_precision(reason="bf16 qk"):
                    nc.vector.tensor_copy(out=qkT[fc][:, s0:s0 + sw], in_=ps)
        return go

    def unit_v(mc):
        def go():
            c0, cw = CHUNKS[mc]
            cwp = CWP[mc]
            ps = ps_a.tile([128, NH, HD], F32, tag="psa", name="pswork")
            for ec in range(4):
                nc.tensor.matmul(
                    out=ps[:cwp],
                    lhsT=inputsT[ec][:, c0:c0 + cwp],
                    rhs=qkvwT[ec][:, 2 * E:3 * E],
                    start=(ec == 0), stop=(ec == 3),
                )
            with nc.allow_low_precision(reason="bf16 v"):
                nc.vector.tensor_copy(out=v_sb[mc][:cw, :, 0:HD], in_=ps[:cw])
            nc.vector.memset(v_sb[mc][:cw, :, HD:HD + 1], 1.0)
        return go

    def unit_scores_pair(q, mc):
        # heads 2q (PE rows 0:64) and 2q+1 (rows 64:128): the two matmuls of a
        # span are emitted adjacently so their disjoint row groups run
        # concurrently on the array.
        def go():
            c0, cw = CHUNKS[mc]
            cwp = CWP[mc]
            kt = qkT[4 + q]
            qt = qkT[q]
            ets = []
            pss = []
            for hh in range(2):
                et = expool.tile([128, N1P], BF16, tag="expT", name="expT")
                expT[(2 * q + hh, mc)] = et
                ets.append(et)
                pss.append(ps_big.tile([128, N1P], F32, tag="big", name="ps_sc"))
            for s0, sw in SPANS:
                for hh in range(2):
                    hb = 64 * hh
                    nc.tensor.matmul(
                        out=pss[hh][:cwp, s0:s0 + sw],
                        lhsT=kt[hb:hb + 64, c0:c0 + cwp],
                        rhs=qt[hb:hb + 64, s0:s0 + sw],
                        start=True, stop=True,
                    )
            for hh in range(2):
                nc.scalar.activation(
                    out=ets[hh][:cwp, :], in_=pss[hh][:cwp, :],
                    func=AFT.Exp, scale=rtau[:cwp],
                )
        return go

    # interleave so scores (ACT exp) work spreads across the whole stream
    units = []
    for q in range(4):
        units.append(unit_qkvT(q))
        units.append(unit_qkvT(4 + q))
        units.append(unit_v(q))
        for mc in range(5):
            units.append(unit_scores_pair(q, mc))
    units.append(unit_v(4))

    # ---- is_one computation (psum -> multiplicative mask) ----
    ps_m = [None, None]

    def emit_isone(i):
        tmp = spool.tile([128, NCLS], F32, tag="isotmp", name="isotmp")
        nc.scalar.activation(out=tmp, in_=ps_m[i], func=AFT.Square, bias=neg1)
        nc.scalar.activation(out=isone[i], in_=tmp, func=AFT.Relu, scale=-1.0, bias=1.0)

    def emit_isoT(i):
        # transpose is_one -> isoT (c on partitions); half i fills columns
        # i*128..i*128+128
        for j in range(3):
            cw = 57 if j == 2 else 128
            pst = ps_a.tile([128, 128], BF16, tag="psa", name="pswork_t")
            nc.tensor.transpose(out=pst[:cw, :], in_=isone[i][:, j * 128:j * 128 + cw],
                                identity=ident_sb)
            nc.vector.tensor_copy(out=isoT[j][:cw, i * 128:(i + 1) * 128], in_=pst[:cw, :])

    # ---- the mask stream: fp8 tiles of [128 rows, w cols x 320c].
    # DoubleRow pairs adjacent image columns; all matmuls of a row-block
    # accumulate the patch sum into ps_m[rt][s, c].
    ui = 0
    last_rt = -1
    for idx, (rt, c0, w) in enumerate(TILES):
        if rt != last_rt:
            ps_m[rt] = ps_mask.tile([128, NCLS], F32, tag="psmask", name="psmask")
            last_rt = rt
        t = pre.pop(idx, None)
        if t is None:
            t = mask_tile_dma(idx)
        first_tile = (c0 == 0)
        last_tile = (c0 + w == IMG)
        for wi in range(w // 16):
            wg = c0 // 16 + wi
            for jp in range(8):
                nc.tensor.matmul(
                    out=ps_m[rt],
                    lhsT=bd_sb[:, wg, :, :],
                    rhs=t[:, wi, jp, :, :NCLS],
                    start=(first_tile and wi == 0 and jp == 0),
                    stop=(last_tile and wi == w // 16 - 1 and jp == 7),
                    perf_mode=DR,
                )
        for _ in range(FILLER[idx]):
            pf = ps_a.tile([128, 512], F32, tag="psa", name="pf")
            nc.tensor.matmul(out=pf, lhsT=scr[:, 0:128], rhs=scr[:, 128:640],
                             start=True, stop=True)
        if last_tile:
            # is_one first so its ACT ops aren't queued behind the units' exps
            emit_isone(rt)
        for _ in range(BUDGET[idx]):
            if ui < len(units):
                units[ui]()
                ui += 1
        if last_tile:
            emit_isoT(rt)
    while ui < len(units):
        units[ui]()
        ui += 1
    mctx.close()
    ps_nrm = ctx.enter_context(tc.tile_pool(name="ps_nrm", bufs=1, space="PSUM"))

    # ---- mask-mult + attn@v + normalize (head pairs) ----
    # Per head: multiply expT by the mask (color-key side on gpsimd, patch-key
    # side on DVE), attn@v into a 2-bank psum (ones column of v gives the
    # softmax denominator in row 64), evacuate rows 0..63 to the head-pair
    # outP tile via ACT and the denominator row straight to partition 32*h4 of
    # the group's den4 tile. One DVE reciprocal per 4 heads; the PE broadcasts
    # each head's recip row into its 64-partition half of a [128,570] psum and
    # one DVE mul normalizes the pair in place.
    outP = [opool.tile([128, N1P], BF16, tag="outP", name="outP") for _ in range(4)]
    for g in range(2):
        for h4 in range(4):
            h = g * 4 + h4
            for mc in range(5):
                c0, cw = CHUNKS[mc]
                et = expT[(h, mc)]
                if mc == 0:
                    nc.gpsimd.tensor_mul(
                        out=et[:cw, SEQ:N1], in0=et[:cw, SEQ:N1], in1=isone[mc])
                elif mc == 1:
                    nc.vector.tensor_mul(
                        out=et[:cw, SEQ:N1], in0=et[:cw, SEQ:N1], in1=isone[mc])
                else:
                    nc.vector.tensor_mul(
                        out=et[:cw, 0:SEQ], in0=et[:cw, 0:SEQ], in1=isoT[mc - 2][:cw, :])
            pso = ps_big.tile([65, N1P], F32, tag="big", name="psout")
            for s0, sw in SPANS:
                for mc in range(5):
                    c0, cw = CHUNKS[mc]
                    nc.tensor.matmul(
                        out=pso[:, s0:s0 + sw],
                        lhsT=v_sb[mc][:cw, h, :],
                        rhs=expT[(h, mc)][:cw, s0:s0 + sw],
                        start=(mc == 0), stop=(mc == 4),
                    )
            with nc.allow_low_precision(reason="bf16 evac"):
                nc.scalar.activation(
                    out=outP[h // 2][64 * (h % 2):64 * (h % 2) + 64, :],
                    in_=pso[0:64, :], func=AFT.Copy)
            nc.scalar.activation(
                out=den4[g][32 * h4:32 * h4 + 1, :],
                in_=pso[64:65, :], func=AFT.Copy)
        nc.vector.reciprocal_approx_fast(out=drec_f, in_=den4[g])
        with nc.allow_low_precision(reason="bf16 recip"):
            nc.vector.tensor_copy(out=drec[g], in_=drec_f)
        for jj in range(2):
            pairidx = 2 * g + jj
            psb = ps_nrm.tile([128, N1P], F32, tag="psb", name="psb")
            # the pair's two row groups (32*h4 vs 32*h4+32) are disjoint, so
            # emitting the two heads' matmuls adjacently per span runs them
            # concurrently on the array
            for s0, sw in SPANS:
                for hh in range(2):
                    h4 = 2 * jj + hh
                    nc.tensor.matmul(
                        out=psb[64 * hh:64 * hh + 64, s0:s0 + sw],
                        lhsT=ones_sb[32 * h4:32 * h4 + 1, :],
                        rhs=drec[g][32 * h4:32 * h4 + 1, s0:s0 + sw],
                        start=True, stop=True,
                        tile_position=(32 * h4, 64 * hh),
                    )
            with nc.allow_low_precision(reason="in-place normalize"):
                nc.vector.tensor_mul(out=outP[pairidx], in0=outP[pairidx], in1=psb)

    # ---- o_proj + bias + store ----
    for mc in range(5):
        c0, cw = CHUNKS[mc]
        cwp = CWP[mc]
        psf = ps_a.tile([128, E], F32, tag="psa", name="psf")
        for j in range(4):
            nc.tensor.matmul(
                out=psf[:cwp, :],
                lhsT=outP[j][:, c0:c0 + cwp],
                rhs=owP[:, j, :],
                start=(j == 0), stop=(j == 3),
            )
        fin = spool.tile([128, E], BF16, tag="fin", name="fin")
        nc.vector.tensor_add(out=fin[:cw, :], in0=psf[:cw, :], in1=ob_bc[:cw, :])
        nc.sync.dma_start(out=d_out[c0:c0 + cw, :], in_=fin[:cw, :])

    ctx.close()


def _constants():
    # block-diag: bd[w][r, s'] = 1 iff s' == (r//16)*16 + w; duplicated in
    # pairs for DoubleRow (both elements of a column pair share the map).
    # Packed host-side as [128 partitions, 16*2*128] so the load is one
    # clean 4KB-per-partition transfer.
    bd = np.zeros((128, 16, 2, 128), dtype=np.float32)
    r = np.arange(128)
    for w in range(16):
        bd[r, w, 0, (r // 16) * 16 + w] = 1.0
        bd[r, w, 1, (r // 16) * 16 + w] = 1.0
    ident = np.eye(128, dtype=ml_dtypes.bfloat16)
    return bd.reshape(128, 16 * 256).astype(ml_dtypes.float8_e4m3), ident


def kernel(x, colors, mask, qkv_w, o_w, o_b, tau):
    global LAST_RESULT
    if "nc" not in _CACHED:
        _CACHED["nc"] = _build_program()
    nc = _CACHED["nc"]

    bd, ident = _constants()
    # pack weight layouts to match SBUF tiles exactly: [part, chunk, col]
    qkv_wT = np.asarray(qkv_w, dtype=np.float32).T.astype(ml_dtypes.bfloat16)
    qkv_wT = np.ascontiguousarray(
        qkv_wT.reshape(4, 128, 3 * E).transpose(1, 0, 2)).reshape(128, 4 * 3 * E)
    # o_w as head-pair blocks: pair j rows 0:64 = head 2j, 64:128 = head 2j+1
    o_wT = np.asarray(o_w, dtype=np.float32).T.astype(ml_dtypes.bfloat16)
    o_wP = np.ascontiguousarray(
        o_wT.reshape(4, 128, E).transpose(1, 0, 2)).reshape(128, 4 * E)
    o_b2 = np.asarray(o_b, dtype=np.float32).reshape(1, E)
    tau2 = np.asarray(tau, dtype=np.float32).reshape(1, 1)

    # mask values are exactly 0.0/1.0 -> cast to fp8 is lossless and quarters
    # the HBM stream; pad the c dim to 320 so DoubleRow pair strides are
    # 16B-aligned
    m8 = np.zeros((B, IMG, IMG, NCP), dtype=ml_dtypes.float8_e4m3)
    m8[..., :NCLS] = np.asarray(mask, dtype=np.float32).astype(ml_dtypes.float8_e4m3)

    in_maps = []
    for b in range(B):
        xTf = np.concatenate([np.asarray(x[b]), np.asarray(colors[b])],
                             axis=0).T.astype(ml_dtypes.bfloat16)
        xT = np.zeros((128, 4, N1P), dtype=ml_dtypes.bfloat16)
        xT[:, :, :N1] = xTf.reshape(4, 128, N1).transpose(1, 0, 2)
        xT = xT.reshape(128, 4 * N1P)
        mb = m8[b].reshape(IMG, IMG * NCP)
        in_maps.append({
            "xT": xT, "mask": mb, "qkv_wT": qkv_wT, "o_wP": o_wP,
            "o_b": o_b2, "tau": tau2, "bd": bd, "ident": ident,
        })

    res = run_bass_kernel_spmd(nc, in_maps, list(range(B)))
    LAST_RESULT = res
    out = np.stack([res.results[i]["out"] for i in range(B)]).astype(np.float32)
    return out
